# revision 4
# baseline (speedup 1.0000x reference)
"""Trainium2 Bass kernel for a debiased GRU cell.

Computation (per batch row):
    r   = sigmoid(W_r @ [x; h] + b_r)
    u   = sigmoid(W_u @ [x; h] + b_u)
    hh  = tanh(W_h @ [x_int; r*h] + b_h)
    s   = score * u
    out = (1 - s) * hh + s * h

Strategy: data-parallel over 8 cores (8192 rows each). On-chip layout is
feature-major ([H, batch]) so activations never need an on-chip transpose
(host supplies x.T / h.T), gate biases fuse into the ACT engine's
per-partition bias operand, and matmuls run with full K/M=128, N=512 tiles.

All of a block's inputs are packed on the host into ONE contiguous
per-partition byte record ([128, RECB] u8, one DMA per block, 128
descriptors) because HWDGE descriptor generation costs ~630 ns per DMA
instruction serialized across all queues — with per-tensor loads it was
the top bottleneck. On-chip the record is sliced via bitcast views.

Variants (precision tuned against the 2e-2 rel-err budget; fp32 sim errs):
  fp32r — everything fp32 (rel err ~1.5e-4). PE floor ~110 us/core.
  bf16  — bf16 DMA + matmul operands + elementwise (~4e-3). Halves HBM
          traffic and doubles DVE throughput; PE floor unchanged.
  fp8g  — gate matmuls in fp8e4 with perf_mode=DoubleRow (2 MACs/cell/cyc),
          h-matmul + elementwise in bf16 (~1.2e-2). Cuts PE time ~1.5x.
Gate weights are pre-scaled by SW=16 on the host so fp8e4 stays in its
normal range; the ACT sigmoid un-scales via its fused scale operand.
"""

import os

import numpy as np

import concourse.bacc as bacc
import concourse.bass as bass
import concourse.mybir as mybir
import concourse.tile as tile
from concourse.bass_utils import run_bass_kernel_spmd

B = 65536
I = 256
H = 256
NCORES = 8
BC = B // NCORES  # rows per core
NB = 512          # batch columns per block (max fp32 matmul free dim)
NBLK = BC // NB   # 16
FP32 = mybir.dt.float32
BF16 = mybir.dt.bfloat16
F8E4 = mybir.dt.float8e4
U8 = mybir.dt.uint8
AF = mybir.ActivationFunctionType
SW = 16.0  # fp8 gate-weight prescale

VARIANT = "fp8g"

# per-partition byte record layout per block: [x-gate, h-gate, x-h, h-ew]
_REC = {
    # (gate x bytes, gate h bytes, h-matmul x bytes, elementwise h bytes)
    "fp32r": (4 * NB * 4, 2 * NB * 4, 0, 0),  # x/h fp32 serve both uses
    "bf16": (4 * NB * 2, 2 * NB * 2, 0, 0),
    "fp8g": (4 * NB * 1, 2 * NB * 1, 2 * NB * 2, 2 * NB * 2),
}

_NC_CACHE = {}


def _build_nc(variant=VARIANT, reps=1, loop=None,
              pg_bufs=6, ph_bufs=2, in_bufs=4, work_bufs=3,
              out_queue="scalar"):
    nc = bacc.Bacc(
        "TRN2",
        target_bir_lowering=False,
        debug=False,
        enable_asserts=False,
    )

    # float32r streams fp32 bits through the PE at full rate (1 cycle/row
    # vs 4 for plain fp32); bit layout is identical to fp32.
    MDT = {"fp32r": mybir.dt.float32r, "bf16": BF16, "fp8g": BF16}[variant]
    GDT = F8E4 if variant == "fp8g" else MDT  # gate-matmul operand dtype
    EDT = FP32 if variant == "fp32r" else BF16  # elementwise dtype
    fp8 = variant == "fp8g"
    szs = _REC[variant]
    offs = np.cumsum([0] + list(szs))
    RECB = int(offs[-1])

    blkin = nc.dram_tensor("bi", [NBLK, 128, RECB], U8, kind="ExternalInput")
    sc = nc.dram_tensor("sc", [1, NBLK * NB], EDT, kind="ExternalInput")
    if fp8:
        wg = nc.dram_tensor("wg", [128, 12, 2, 128], F8E4, kind="ExternalInput")
    else:
        wg = nc.dram_tensor("wg", [128, 24 * 128], MDT, kind="ExternalInput")
    wh = nc.dram_tensor("wh", [128, 8 * 128], MDT, kind="ExternalInput")
    bg = nc.dram_tensor("bg", [128, 4], FP32, kind="ExternalInput")
    bh = nc.dram_tensor("bh", [128, 2], FP32, kind="ExternalInput")
    outT = nc.dram_tensor("outT", [H, BC], EDT, kind="ExternalOutput")
    outTr = outT.rearrange("(m p) (b n) -> b p m n", p=128, n=NB)

    with tile.TileContext(nc) as tc:
        with (
            tc.tile_pool(name="const", bufs=1) as cpool,
            tc.tile_pool(name="xin", bufs=in_bufs) as xpool,
            tc.tile_pool(name="sin", bufs=in_bufs) as spool,
            tc.tile_pool(name="gates", bufs=work_bufs) as gpool,
            tc.tile_pool(name="work", bufs=work_bufs) as wpool,
            tc.tile_pool(name="outp", bufs=work_bufs) as opool,
            tc.tile_pool(name="psg", bufs=pg_bufs, space=bass.MemorySpace.PSUM) as pgpool,
            tc.tile_pool(name="psh", bufs=ph_bufs, space=bass.MemorySpace.PSUM) as phpool,
        ):
            # Gate weights split per gate so the first gate chain only waits
            # on its own slice, not the full weight load.
            if fp8:
                wg_sb = cpool.tile([128, 12, 2, 128], F8E4)
                for gi in range(4):
                    nc.sync.dma_start(wg_sb[:, gi * 3:(gi + 1) * 3],
                                      wg[:, gi * 3:(gi + 1) * 3])
            else:
                wg_sb = cpool.tile([128, 24 * 128], MDT)
                for gi in range(4):
                    nc.sync.dma_start(wg_sb[:, gi * 768:(gi + 1) * 768],
                                      wg[:, gi * 768:(gi + 1) * 768])
            bg_sb = cpool.tile([128, 4], FP32)
            nc.sync.dma_start(bg_sb[:], bg[:])
            wh_sb = cpool.tile([128, 8 * 128], MDT)
            nc.sync.dma_start(wh_sb[:], wh[:])
            bh_sb = cpool.tile([128, 2], FP32)
            nc.sync.dma_start(bh_sb[:], bh[:])
            srow_all = cpool.tile([1, NBLK * NB], EDT)
            nc.sync.dma_start(srow_all[:], sc[:])
            # One broadcast of the whole score row for all 16 blocks; e2
            # reads it through a stride-0 view along the m dim.
            sbc_all = cpool.tile([128, NBLK * NB], EDT)
            nc.gpsimd.partition_broadcast(sbc_all[:], srow_all[:])

            def load_block(g):
                """One DMA for the whole block record + score broadcast."""
                blk = xpool.tile([128, RECB], U8, tag="blk")
                nc.sync.dma_start(blk[:], blkin[g])

                def view(i, dt, kc):
                    return (blk[:, int(offs[i]):int(offs[i + 1])]
                            .bitcast(dt).rearrange("p (k n) -> p k n", n=NB))
                xg = view(0, GDT, 4)          # [128, 4, NB] gate x operand
                hg = view(1, GDT, 2)          # [128, 2, NB] gate h operand
                if fp8:
                    xh = view(2, BF16, 2)     # [128, 2, NB] h-matmul int_emb
                    he = view(3, BF16, 2)     # [128, 2, NB] elementwise h
                else:
                    xh = xg                   # first 2 chunks reused
                    he = hg
                sbc = spool.tile([128, 2, NB], EDT, tag="sbc")
                srow = srow_all[:, g * NB:(g + 1) * NB]
                nc.gpsimd.partition_broadcast(sbc[:, 0, :], srow)
                nc.gpsimd.partition_broadcast(sbc[:, 1, :], srow)
                og = opool.tile([128, 2, NB], EDT, tag="o")
                return dict(g=g, xg=xg, hg=hg, xh=xh, he=he, sbc=sbc, og=og)

            def emit_gates(st):
                """Gate matmuls + sigmoids + r*h (+ e2, A) for block st."""
                b = st["g"]
                pgs = [pgpool.tile([128, NB], FP32, tag="pg", name=f"pg{b}_{i}")
                       for i in range(4)]
                if fp8:
                    for gi in range(4):  # r0, r1, u0, u1
                        for kp in range(3):
                            rhs = (st["xg"][:, 2 * kp:2 * kp + 2, :] if kp < 2
                                   else st["hg"][:, 0:2, :])
                            nc.tensor.matmul(
                                pgs[gi][:],
                                wg_sb[:, gi * 3 + kp],
                                rhs,
                                start=(kp == 0),
                                stop=(kp == 2),
                                perf_mode=mybir.MatmulPerfMode.DoubleRow,
                            )
                else:
                    for gi in range(4):
                        for k in range(6):
                            act = (st["xg"][:, k, :] if k < 4
                                   else st["hg"][:, k - 4, :])
                            c = gi * 6 + k
                            nc.tensor.matmul(
                                pgs[gi][:],
                                wg_sb[:, c * 128:(c + 1) * 128],
                                act,
                                start=(k == 0),
                                stop=(k == 5),
                            )
                r = gpool.tile([128, 2, NB], EDT, tag="r")
                u = gpool.tile([128, 2, NB], EDT, tag="u")
                gsc = 1.0 / SW if fp8 else 1.0
                for m in range(2):
                    nc.scalar.activation(
                        r[:, m, :], pgs[m][:],
                        AF.Sigmoid, bias=bg_sb[:, m:m + 1], scale=gsc,
                    )
                    nc.scalar.activation(
                        u[:, m, :], pgs[2 + m][:],
                        AF.Sigmoid, bias=bg_sb[:, 2 + m:3 + m], scale=gsc,
                    )
                rh = wpool.tile([128, 2, NB], MDT, tag="rh")
                nc.vector.tensor_mul(rh[:], r[:], st["he"][:])
                # e2 = score*u and A = h*e2 only depend on the gate phase, so
                # they run here, off the post-tanh critical tail.
                e2 = wpool.tile([128, 2, NB], EDT, tag="e2")
                nc.vector.tensor_mul(e2[:], u[:], st["sbc"][:])
                A = wpool.tile([128, 2, NB], EDT, tag="A")
                nc.vector.tensor_mul(A[:], st["he"][:], e2[:])
                st.update(rh=rh, e2=e2, A=A)
                return st

            def emit_h(st):
                """h_hat matmul + tanh + final combine + store for block b."""
                b = st["g"]
                phs = [phpool.tile([128, NB], FP32, tag="ph", name=f"ph{b}_{i}")
                       for i in range(2)]
                for m in range(2):
                    for k in range(4):
                        act = (st["xh"][:, k, :] if k < 2
                               else st["rh"][:, k - 2, :])
                        c = m * 4 + k
                        nc.tensor.matmul(
                            phs[m][:],
                            wh_sb[:, c * 128:(c + 1) * 128],
                            act,
                            start=(k == 0),
                            stop=(k == 3),
                        )
                hhat = wpool.tile([128, 2, NB], EDT, tag="hhat")
                for m in range(2):
                    nc.scalar.activation(
                        hhat[:, m, :], phs[m][:],
                        AF.Tanh, bias=bh_sb[:, m:m + 1]
                    )
                # out = A - (e2-1)*hh  ==  hh + e2*(h - hh), with A = h*e2
                C = wpool.tile([128, 2, NB], EDT, tag="C")
                nc.vector.scalar_tensor_tensor(
                    C[:], st["e2"][:], 1.0, hhat[:],
                    op0=mybir.AluOpType.subtract, op1=mybir.AluOpType.mult,
                )
                nc.vector.tensor_sub(st["og"][:], st["A"][:], C[:])
                # store on the ACT HWDGE ring so it doesn't queue behind
                # the input loads on the SP ring
                out_eng = nc.scalar if out_queue == "scalar" else nc.sync
                out_eng.dma_start(outTr[b], st["og"][:])

            # Software-pipelined emission: block b's h-chain is emitted after
            # block b+1's gate matmuls so the PE never waits on the r*h
            # elementwise product. reps>1 repeats the whole pass (same
            # output) — used only for slope-based timing in bench.py.
            def emit_pass():
                prev = None
                for _rep in range(reps):
                    for g in range(NBLK):
                        st = emit_gates(load_block(g))
                        if prev is not None:
                            emit_h(prev)
                        prev = st
                emit_h(prev)

            if loop is None:
                emit_pass()
            else:
                # bench-only: repeat the whole pass `loop` times inside one
                # NEFF execution for slope-based timing.
                with tc.For_i(0, loop, 1):
                    emit_pass()

    nc.compile()
    return nc


def _get_nc():
    if "nc" not in _NC_CACHE:
        _NC_CACHE["nc"] = _build_nc()
    return _NC_CACHE["nc"]


def _chunk_bytes(a, dt, kc):
    """[kc*128, BC] fp32 -> [NBLK, 128, kc*NB*size(dt)] u8, layout [b,p,k,n]."""
    v = np.ascontiguousarray(
        a.reshape(kc, 128, NBLK, NB).transpose(2, 1, 0, 3)).astype(dt)
    return v.reshape(NBLK, 128, -1).view(np.uint8)


def _pack_weights(W_r, W_u, W_h, b_r, b_u, b_h, variant):
    gb = np.empty((4, 6, 128, 128), np.float32)
    for gi in range(4):
        W = W_r if gi < 2 else W_u
        m = gi % 2
        for k in range(6):
            gb[gi, k] = W[m * 128:(m + 1) * 128, k * 128:(k + 1) * 128].T
    if variant == "fp8g":
        wg = np.ascontiguousarray(
            gb.reshape(24, 128, 128).transpose(1, 0, 2).reshape(128, 12, 2, 128)
            * SW
        ).astype(mybir.dt.np(F8E4))
    else:
        dt = np.float32 if variant == "fp32r" else mybir.dt.np(BF16)
        wg = np.ascontiguousarray(
            gb.reshape(24, 128, 128).transpose(1, 0, 2).reshape(128, 24 * 128)
        ).astype(dt)
    hdt = np.float32 if variant == "fp32r" else mybir.dt.np(BF16)
    whb = np.empty((2, 4, 128, 128), np.float32)
    for m in range(2):
        for k in range(4):
            whb[m, k] = W_h[m * 128:(m + 1) * 128, k * 128:(k + 1) * 128].T
    wh = np.ascontiguousarray(
        whb.reshape(8, 128, 128).transpose(1, 0, 2).reshape(128, 8 * 128)
    ).astype(hdt)
    bgp = np.stack([b_r[:128], b_r[128:], b_u[:128], b_u[128:]], axis=1)
    bhp = np.stack([b_h[:128], b_h[128:]], axis=1)
    return wg, wh, np.ascontiguousarray(bgp), np.ascontiguousarray(bhp)


def _make_in_maps(inputs, h_prev, attention_score, W_r, b_r, W_u, b_u, W_h, b_h,
                  variant=VARIANT):
    inputs = np.asarray(inputs, np.float32)
    h_prev = np.asarray(h_prev, np.float32)
    attention_score = np.asarray(attention_score, np.float32)
    wg, wh, bgp, bhp = _pack_weights(
        np.asarray(W_r, np.float32), np.asarray(W_u, np.float32),
        np.asarray(W_h, np.float32), np.asarray(b_r, np.float32),
        np.asarray(b_u, np.float32), np.asarray(b_h, np.float32), variant,
    )
    nb16 = mybir.dt.np(BF16)
    nf8 = mybir.dt.np(F8E4)
    sdt = np.float32 if variant == "fp32r" else nb16
    in_maps = []
    for c in range(NCORES):
        sl = slice(c * BC, (c + 1) * BC)
        xTc = np.ascontiguousarray(inputs[sl].T)
        hTc = np.ascontiguousarray(h_prev[sl].T)
        if variant == "fp8g":
            parts = [
                _chunk_bytes(xTc, nf8, 4),
                _chunk_bytes(hTc, nf8, 2),
                _chunk_bytes(xTc[:I], nb16, 2),
                _chunk_bytes(hTc, nb16, 2),
            ]
        else:
            gdt = np.float32 if variant == "fp32r" else nb16
            parts = [_chunk_bytes(xTc, gdt, 4), _chunk_bytes(hTc, gdt, 2)]
        bi = np.ascontiguousarray(np.concatenate(parts, axis=2))
        in_maps.append({
            "bi": bi,
            "sc": np.ascontiguousarray(
                attention_score[sl].reshape(1, NBLK * NB)).astype(sdt),
            "wg": wg, "wh": wh, "bg": bgp, "bh": bhp,
        })
    return in_maps


def _run(in_maps, trace=False, **kwargs):
    try:
        return run_bass_kernel_spmd(
            _get_nc(), in_maps, core_ids=list(range(NCORES)), trace=trace, **kwargs
        )
    except ModuleNotFoundError:
        # A global BASS_TRACE=1 enables the NTFF trace path, which needs
        # antenv.axon_hooks; on images without it, retry untraced.
        had = os.environ.get("BASS_NEVER_TRACE")
        os.environ["BASS_NEVER_TRACE"] = "1"
        try:
            return run_bass_kernel_spmd(
                _get_nc(), in_maps, core_ids=list(range(NCORES)), trace=False,
                **kwargs
            )
        finally:
            if had is None:
                del os.environ["BASS_NEVER_TRACE"]
            else:
                os.environ["BASS_NEVER_TRACE"] = had


def _gather(results):
    out = np.empty((B, H), np.float32)
    for c in range(NCORES):
        out[c * BC:(c + 1) * BC] = results[c]["outT"].T.astype(np.float32)
    return out


def kernel(**inputs):
    res = _run(_make_in_maps(**inputs), trace=False)
    return _gather(res.results)


# revision 10
# speedup vs baseline: 8.9435x; 8.9435x over previous
"""Trainium2 Bass kernel for a debiased GRU cell.

Computation (per batch row):
    r   = sigmoid(W_r @ [x; h] + b_r)
    u   = sigmoid(W_u @ [x; h] + b_u)
    hh  = tanh(W_h @ [x_int; r*h] + b_h)
    s   = score * u
    out = (1 - s) * hh + s * h

Strategy: data-parallel over 8 cores (8192 rows each). On-chip layout is
feature-major ([H, batch]) so activations never need an on-chip transpose
(host supplies x.T / h.T), gate biases fuse into the ACT engine's
per-partition bias operand, and matmuls run with full K/M=128, N=512 tiles.

All of a block's inputs are packed on the host into ONE contiguous
per-partition byte record ([128, RECB] u8, one DMA per block, 128
descriptors) because HWDGE descriptor generation costs ~630 ns per DMA
instruction serialized across all queues — with per-tensor loads it was
the top bottleneck. On-chip the record is sliced via bitcast views.

Variants (precision tuned against the 2e-2 rel-err budget; fp32 sim errs):
  fp32r — everything fp32 (rel err ~1.5e-4). PE floor ~110 us/core.
  bf16  — bf16 DMA + matmul operands + elementwise (~4e-3). Halves HBM
          traffic and doubles DVE throughput; PE floor unchanged.
  fp8g  — gate matmuls in fp8e4 with perf_mode=DoubleRow (2 MACs/cell/cyc),
          h-matmul + elementwise in bf16 (~1.2e-2). Cuts PE time ~1.5x.
Gate weights are pre-scaled by SW=16 on the host so fp8e4 stays in its
normal range; the ACT sigmoid un-scales via its fused scale operand.
"""

import os

import numpy as np

import concourse.bacc as bacc
import concourse.bass as bass
import concourse.mybir as mybir
import concourse.tile as tile
from concourse.bass_utils import run_bass_kernel_spmd

B = 65536
I = 256
H = 256
NCORES = 8
BC = B // NCORES  # rows per core
NB = 512          # batch columns per block (max fp32 matmul free dim)
NBLK = BC // NB   # 16
FP32 = mybir.dt.float32
BF16 = mybir.dt.bfloat16
F8E4 = mybir.dt.float8e4
U8 = mybir.dt.uint8
AF = mybir.ActivationFunctionType
SW = 16.0  # fp8 gate-weight prescale

VARIANT = "fp8g"

# per-partition byte record layout per block: [x-gate, h-gate, x-h, h-ew]
_REC = {
    # (gate x bytes, gate h bytes, h-matmul x bytes, elementwise h bytes)
    "fp32r": (4 * NB * 4, 2 * NB * 4, 0, 0),  # x/h fp32 serve both uses
    "bf16": (4 * NB * 2, 2 * NB * 2, 0, 0),
    "fp8g": (4 * NB * 1, 2 * NB * 1, 2 * NB * 2, 2 * NB * 2),
}

_NC_CACHE = {}


def _build_nc(variant=VARIANT, reps=1, loop=None,
              pg_bufs=6, ph_bufs=2, in_bufs=4, work_bufs=3,
              out_queue="sync"):
    nc = bacc.Bacc(
        "TRN2",
        target_bir_lowering=False,
        debug=False,
        enable_asserts=False,
    )

    # float32r streams fp32 bits through the PE at full rate (1 cycle/row
    # vs 4 for plain fp32); bit layout is identical to fp32.
    MDT = {"fp32r": mybir.dt.float32r, "bf16": BF16, "fp8g": BF16}[variant]
    GDT = F8E4 if variant == "fp8g" else MDT  # gate-matmul operand dtype
    EDT = FP32 if variant == "fp32r" else BF16  # elementwise dtype
    fp8 = variant == "fp8g"
    szs = _REC[variant]
    offs = np.cumsum([0] + list(szs))
    RECB = int(offs[-1])

    blkin = nc.dram_tensor("bi", [NBLK, 128, RECB], U8, kind="ExternalInput")
    sc = nc.dram_tensor("sc", [1, NBLK * NB], EDT, kind="ExternalInput")
    if fp8:
        wg = nc.dram_tensor("wg", [128, 12, 2, 128], F8E4, kind="ExternalInput")
    else:
        wg = nc.dram_tensor("wg", [128, 24 * 128], MDT, kind="ExternalInput")
    wh = nc.dram_tensor("wh", [128, 8 * 128], MDT, kind="ExternalInput")
    bg = nc.dram_tensor("bg", [128, 4], FP32, kind="ExternalInput")
    bh = nc.dram_tensor("bh", [128, 2], FP32, kind="ExternalInput")
    outT = nc.dram_tensor("outT", [H, BC], EDT, kind="ExternalOutput")
    outTr = outT.rearrange("(m p) (b n) -> b p m n", p=128, n=NB)

    with tile.TileContext(nc) as tc:
        with (
            tc.tile_pool(name="const", bufs=1) as cpool,
            tc.tile_pool(name="xin", bufs=in_bufs) as xpool,
            tc.tile_pool(name="sin", bufs=in_bufs) as spool,
            tc.tile_pool(name="gates", bufs=work_bufs) as gpool,
            tc.tile_pool(name="work", bufs=work_bufs) as wpool,
            tc.tile_pool(name="outp", bufs=work_bufs) as opool,
            tc.tile_pool(name="psg", bufs=pg_bufs, space=bass.MemorySpace.PSUM) as pgpool,
            tc.tile_pool(name="psh", bufs=ph_bufs, space=bass.MemorySpace.PSUM) as phpool,
        ):
            # Gate weights split per gate so the first gate chain only waits
            # on its own slice, not the full weight load.
            if fp8:
                wg_sb = cpool.tile([128, 12, 2, 128], F8E4)
                for gi in range(4):
                    nc.sync.dma_start(wg_sb[:, gi * 3:(gi + 1) * 3],
                                      wg[:, gi * 3:(gi + 1) * 3])
            else:
                wg_sb = cpool.tile([128, 24 * 128], MDT)
                for gi in range(4):
                    nc.sync.dma_start(wg_sb[:, gi * 768:(gi + 1) * 768],
                                      wg[:, gi * 768:(gi + 1) * 768])
            bg_sb = cpool.tile([128, 4], FP32)
            nc.sync.dma_start(bg_sb[:], bg[:])
            wh_sb = cpool.tile([128, 8 * 128], MDT)
            nc.sync.dma_start(wh_sb[:], wh[:])
            bh_sb = cpool.tile([128, 2], FP32)
            nc.sync.dma_start(bh_sb[:], bh[:])
            srow_all = cpool.tile([1, NBLK * NB], EDT)
            nc.sync.dma_start(srow_all[:], sc[:])

            def load_block(g):
                """One DMA for the whole block record + score broadcast."""
                blk = xpool.tile([128, RECB], U8, tag="blk")
                nc.sync.dma_start(blk[:], blkin[g])

                def view(i, dt, kc):
                    return (blk[:, int(offs[i]):int(offs[i + 1])]
                            .bitcast(dt).rearrange("p (k n) -> p k n", n=NB))
                xg = view(0, GDT, 4)          # [128, 4, NB] gate x operand
                hg = view(1, GDT, 2)          # [128, 2, NB] gate h operand
                if fp8:
                    xh = view(2, BF16, 2)     # [128, 2, NB] h-matmul int_emb
                    he = view(3, BF16, 2)     # [128, 2, NB] elementwise h
                else:
                    xh = xg                   # first 2 chunks reused
                    he = hg
                # single-row broadcast; e2 reads it through a stride-0 view
                # along the m dim (both output halves share the same scores)
                sbc = spool.tile([128, NB], EDT, tag="sbc")
                nc.gpsimd.partition_broadcast(
                    sbc[:], srow_all[:, g * NB:(g + 1) * NB])
                og = opool.tile([128, 2, NB], EDT, tag="o")
                return dict(g=g, xg=xg, hg=hg, xh=xh, he=he, sbc=sbc, og=og)

            def emit_gates(st):
                """Gate matmuls + sigmoids + r*h (+ e2, A) for block st."""
                b = st["g"]
                pgs = [pgpool.tile([128, NB], FP32, tag="pg", name=f"pg{b}_{i}")
                       for i in range(4)]
                if fp8:
                    for gi in range(4):  # r0, r1, u0, u1
                        for kp in range(3):
                            rhs = (st["xg"][:, 2 * kp:2 * kp + 2, :] if kp < 2
                                   else st["hg"][:, 0:2, :])
                            nc.tensor.matmul(
                                pgs[gi][:],
                                wg_sb[:, gi * 3 + kp],
                                rhs,
                                start=(kp == 0),
                                stop=(kp == 2),
                                perf_mode=mybir.MatmulPerfMode.DoubleRow,
                            )
                else:
                    for gi in range(4):
                        for k in range(6):
                            act = (st["xg"][:, k, :] if k < 4
                                   else st["hg"][:, k - 4, :])
                            c = gi * 6 + k
                            nc.tensor.matmul(
                                pgs[gi][:],
                                wg_sb[:, c * 128:(c + 1) * 128],
                                act,
                                start=(k == 0),
                                stop=(k == 5),
                            )
                r = gpool.tile([128, 2, NB], EDT, tag="r")
                u = gpool.tile([128, 2, NB], EDT, tag="u")
                gsc = 1.0 / SW if fp8 else 1.0
                for m in range(2):
                    nc.scalar.activation(
                        r[:, m, :], pgs[m][:],
                        AF.Sigmoid, bias=bg_sb[:, m:m + 1], scale=gsc,
                    )
                    nc.scalar.activation(
                        u[:, m, :], pgs[2 + m][:],
                        AF.Sigmoid, bias=bg_sb[:, 2 + m:3 + m], scale=gsc,
                    )
                rh = wpool.tile([128, 2, NB], MDT, tag="rh")
                nc.vector.tensor_mul(rh[:], r[:], st["he"][:])
                # e2 = score*u and A = h*e2 only depend on the gate phase, so
                # they run here, off the post-tanh critical tail.
                e2 = wpool.tile([128, 2, NB], EDT, tag="e2")
                nc.vector.tensor_mul(
                    e2[:], u[:],
                    st["sbc"][:].unsqueeze(1).to_broadcast((128, 2, NB)))
                A = wpool.tile([128, 2, NB], EDT, tag="A")
                nc.vector.tensor_mul(A[:], st["he"][:], e2[:])
                st.update(rh=rh, e2=e2, A=A)
                return st

            def emit_h(st):
                """h_hat matmul + tanh + final combine + store for block b."""
                b = st["g"]
                phs = [phpool.tile([128, NB], FP32, tag="ph", name=f"ph{b}_{i}")
                       for i in range(2)]
                for m in range(2):
                    for k in range(4):
                        act = (st["xh"][:, k, :] if k < 2
                               else st["rh"][:, k - 2, :])
                        c = m * 4 + k
                        nc.tensor.matmul(
                            phs[m][:],
                            wh_sb[:, c * 128:(c + 1) * 128],
                            act,
                            start=(k == 0),
                            stop=(k == 3),
                        )
                hhat = wpool.tile([128, 2, NB], EDT, tag="hhat")
                for m in range(2):
                    nc.scalar.activation(
                        hhat[:, m, :], phs[m][:],
                        AF.Tanh, bias=bh_sb[:, m:m + 1]
                    )
                # out = A - (e2-1)*hh  ==  hh + e2*(h - hh), with A = h*e2
                C = wpool.tile([128, 2, NB], EDT, tag="C")
                nc.vector.scalar_tensor_tensor(
                    C[:], st["e2"][:], 1.0, hhat[:],
                    op0=mybir.AluOpType.subtract, op1=mybir.AluOpType.mult,
                )
                nc.vector.tensor_sub(st["og"][:], st["A"][:], C[:])
                # store on the ACT HWDGE ring so it doesn't queue behind
                # the input loads on the SP ring
                out_eng = nc.scalar if out_queue == "scalar" else nc.sync
                out_eng.dma_start(outTr[b], st["og"][:])

            # Software-pipelined emission: block b's h-chain is emitted after
            # block b+1's gate matmuls so the PE never waits on the r*h
            # elementwise product. reps>1 repeats the whole pass (same
            # output) — used only for slope-based timing in bench.py.
            def emit_pass():
                prev = None
                for _rep in range(reps):
                    for g in range(NBLK):
                        st = emit_gates(load_block(g))
                        if prev is not None:
                            emit_h(prev)
                        prev = st
                emit_h(prev)

            if loop is None:
                emit_pass()
            else:
                # bench-only: repeat the whole pass `loop` times inside one
                # NEFF execution for slope-based timing.
                with tc.For_i(0, loop, 1):
                    emit_pass()

    nc.compile()
    return nc


def _get_nc():
    if "nc" not in _NC_CACHE:
        _NC_CACHE["nc"] = _build_nc()
    return _NC_CACHE["nc"]


def _chunk_bytes(a, dt, kc):
    """[kc*128, BC] fp32 -> [NBLK, 128, kc*NB*size(dt)] u8, layout [b,p,k,n]."""
    v = np.ascontiguousarray(
        a.reshape(kc, 128, NBLK, NB).transpose(2, 1, 0, 3)).astype(dt)
    return v.reshape(NBLK, 128, -1).view(np.uint8)


def _pack_weights(W_r, W_u, W_h, b_r, b_u, b_h, variant):
    gb = np.empty((4, 6, 128, 128), np.float32)
    for gi in range(4):
        W = W_r if gi < 2 else W_u
        m = gi % 2
        for k in range(6):
            gb[gi, k] = W[m * 128:(m + 1) * 128, k * 128:(k + 1) * 128].T
    if variant == "fp8g":
        wg = np.ascontiguousarray(
            gb.reshape(24, 128, 128).transpose(1, 0, 2).reshape(128, 12, 2, 128)
            * SW
        ).astype(mybir.dt.np(F8E4))
    else:
        dt = np.float32 if variant == "fp32r" else mybir.dt.np(BF16)
        wg = np.ascontiguousarray(
            gb.reshape(24, 128, 128).transpose(1, 0, 2).reshape(128, 24 * 128)
        ).astype(dt)
    hdt = np.float32 if variant == "fp32r" else mybir.dt.np(BF16)
    whb = np.empty((2, 4, 128, 128), np.float32)
    for m in range(2):
        for k in range(4):
            whb[m, k] = W_h[m * 128:(m + 1) * 128, k * 128:(k + 1) * 128].T
    wh = np.ascontiguousarray(
        whb.reshape(8, 128, 128).transpose(1, 0, 2).reshape(128, 8 * 128)
    ).astype(hdt)
    bgp = np.stack([b_r[:128], b_r[128:], b_u[:128], b_u[128:]], axis=1)
    bhp = np.stack([b_h[:128], b_h[128:]], axis=1)
    return wg, wh, np.ascontiguousarray(bgp), np.ascontiguousarray(bhp)


def _make_in_maps(inputs, h_prev, attention_score, W_r, b_r, W_u, b_u, W_h, b_h,
                  variant=VARIANT):
    inputs = np.asarray(inputs, np.float32)
    h_prev = np.asarray(h_prev, np.float32)
    attention_score = np.asarray(attention_score, np.float32)
    wg, wh, bgp, bhp = _pack_weights(
        np.asarray(W_r, np.float32), np.asarray(W_u, np.float32),
        np.asarray(W_h, np.float32), np.asarray(b_r, np.float32),
        np.asarray(b_u, np.float32), np.asarray(b_h, np.float32), variant,
    )
    nb16 = mybir.dt.np(BF16)
    nf8 = mybir.dt.np(F8E4)
    sdt = np.float32 if variant == "fp32r" else nb16
    in_maps = []
    for c in range(NCORES):
        sl = slice(c * BC, (c + 1) * BC)
        xTc = np.ascontiguousarray(inputs[sl].T)
        hTc = np.ascontiguousarray(h_prev[sl].T)
        if variant == "fp8g":
            parts = [
                _chunk_bytes(xTc, nf8, 4),
                _chunk_bytes(hTc, nf8, 2),
                _chunk_bytes(xTc[:I], nb16, 2),
                _chunk_bytes(hTc, nb16, 2),
            ]
        else:
            gdt = np.float32 if variant == "fp32r" else nb16
            parts = [_chunk_bytes(xTc, gdt, 4), _chunk_bytes(hTc, gdt, 2)]
        bi = np.ascontiguousarray(np.concatenate(parts, axis=2))
        in_maps.append({
            "bi": bi,
            "sc": np.ascontiguousarray(
                attention_score[sl].reshape(1, NBLK * NB)).astype(sdt),
            "wg": wg, "wh": wh, "bg": bgp, "bh": bhp,
        })
    return in_maps


def _run(in_maps, trace=False, **kwargs):
    try:
        return run_bass_kernel_spmd(
            _get_nc(), in_maps, core_ids=list(range(NCORES)), trace=trace, **kwargs
        )
    except ModuleNotFoundError:
        # A global BASS_TRACE=1 enables the NTFF trace path, which needs
        # antenv.axon_hooks; on images without it, retry untraced.
        had = os.environ.get("BASS_NEVER_TRACE")
        os.environ["BASS_NEVER_TRACE"] = "1"
        try:
            return run_bass_kernel_spmd(
                _get_nc(), in_maps, core_ids=list(range(NCORES)), trace=False,
                **kwargs
            )
        finally:
            if had is None:
                del os.environ["BASS_NEVER_TRACE"]
            else:
                os.environ["BASS_NEVER_TRACE"] = had


def _gather(results):
    out = np.empty((B, H), np.float32)
    for c in range(NCORES):
        out[c * BC:(c + 1) * BC] = results[c]["outT"].T.astype(np.float32)
    return out


def kernel(**inputs):
    res = _run(_make_in_maps(**inputs), trace=False)
    return _gather(res.results)
